# revision 3
# baseline (speedup 1.0000x reference)
"""Trainium2 Bass kernel for nn_BigramLanguageModel (dense transformer block).

Reference computation (B=2, T=2048, E=1024, V=32000):
    x      = emb_table[X] + pos_table                       # [B,T,E]
    k,q,v  = x@Wk, x@Wq, x@Wv                               # [B,T,E]
    w      = (q @ k^T) / sqrt(E), causal mask (tril)        # [B,T,T]
    w      = softmax(w, axis=1)          # QUIRK: over the *query* axis
    out    = w @ v                                          # [B,T,E]
    logits = out @ Wro + bro                                # [B,T,V]

Sharding: 8 cores = 2 (batch) x 4 (vocab slices of 8000 for the readout
matmul, which dominates FLOPs).  Within each batch group of 4 cores the
attention itself is additionally sharded over *key* tiles (interleaved:
core with vocab index dv owns global k-tiles {dv + 4j}), which keeps the
SPMD program identical across cores: the diagonal-mask variant depends
only on dv (an input), and chunk skip/include rules depend only on the
local tile index j.  Partial attention outputs are AllReduce-summed over
the batch group, then each core runs the readout for its vocab slice.

Device-side layout trick: scores are computed transposed, wT[k,q], so the
softmax-over-q runs along the free axis.  The softmax denominator depends
only on k, so it is folded into V (V' = V/denom[k]) and the attention
output is produced directly in outT[e,q] layout — exactly the lhsT layout
the readout matmul wants.  Causal masking uses the block structure: chunks
with q_end <= k0 are never computed nor read; only the single diagonal
512-chunk per k-tile needs an additive staircase mask.

All matmul operands are bf16 (full PE rate), accumulation fp32 in PSUM.
"""

import sys

if "/opt/trn_rl_repo" not in sys.path:
    sys.path.insert(0, "/opt/trn_rl_repo")

import numpy as np
import ml_dtypes

import concourse.bass as bass
import concourse.tile as tile
from concourse import bacc, mybir
from concourse.bass_utils import run_bass_kernel_spmd

P = 128
B, T, E, VOC = 2, 2048, 1024, 32000
VSPLIT = 4                # vocab splits per batch group
VS = VOC // VSPLIT        # 8000 vocab columns per core
NE = E // P               # 8 embedding partition-tiles
NT = T // P               # 16 token partition-tiles
KL = NT // VSPLIT         # 4 local k-tiles per core (interleaved by dv)
TK = KL * P               # 512 key tokens per core
QCH = 512                 # q chunk width
NQC = T // QCH            # 4
VCH = 500                 # vocab chunk width (<=512 psum bank, 8000 = 16*500)
NVC = VS // VCH           # 16
SCALE = 1.0 / 32.0        # 1/sqrt(E)
MASK_VAL = -960000.0      # additive pre-scale mask; /32 -> -30000 -> exp = 0

BF16 = mybir.dt.bfloat16
F32 = mybir.dt.float32

_CACHE: dict = {}


def _build_program():
    nc = bacc.Bacc("TRN2", target_bir_lowering=False, debug=False, num_devices=8)

    xT_d = nc.dram_tensor("xT", [NE, P, T], BF16, kind="ExternalInput").ap()
    xTk_d = nc.dram_tensor("xTk", [NE, P, TK], BF16, kind="ExternalInput").ap()
    wk_d = nc.dram_tensor("wk", [NE, P, E], BF16, kind="ExternalInput").ap()
    wq_d = nc.dram_tensor("wq", [NE, P, E], BF16, kind="ExternalInput").ap()
    wv_d = nc.dram_tensor("wv", [NE, P, E], BF16, kind="ExternalInput").ap()
    wro_d = nc.dram_tensor("wro", [NE, P, VS], BF16, kind="ExternalInput").ap()
    mask_d = nc.dram_tensor("mask", [P, QCH], F32, kind="ExternalInput").ap()
    out_d = nc.dram_tensor("logits", [NT, P, VS], F32, kind="ExternalOutput").ap()

    Exp = mybir.ActivationFunctionType.Exp
    groups = [[0, 1, 2, 3], [4, 5, 6, 7]]

    with tile.TileContext(nc) as tc:
        from contextlib import ExitStack

        with ExitStack() as root:
            misc = root.enter_context(tc.tile_pool(name="misc", bufs=1))
            psum = root.enter_context(tc.tile_pool(name="psum", bufs=6, space="PSUM"))
            stage = root.enter_context(tc.tile_pool(name="stage", bufs=2))
            dram = root.enter_context(tc.tile_pool(name="dram", bufs=1, space="DRAM"))

            mask_t = misc.tile([P, QCH], F32, tag="mask", name="mask_t")
            nc.sync.dma_start(mask_t[:], mask_d[:])
            parts_t = misc.tile([P, KL, NQC], F32, tag="parts", name="parts_t")
            denom_t = misc.tile([P, KL], F32, tag="denom", name="denom_t")
            recip_t = misc.tile([P, KL], F32, tag="recip", name="recip_t")

            cc_in = dram.tile([NE, P, T], BF16, tag="cci", name="cc_in")
            cc_out = dram.tile([NE, P, T], BF16, tag="cco", name="cc_out")

            # ---- local k/v pools (live through AV phase) ----------------
            pkv = root.enter_context(tc.tile_pool(name="pkv", bufs=1))
            kTl_t = [pkv.tile([P, TK], BF16, tag=f"kTl{i}", name=f"kTl{i}") for i in range(NE)]
            vl_t = [pkv.tile([P, E], BF16, tag=f"vl{i}", name=f"vl{i}") for i in range(KL)]

            pq = ExitStack()
            q_pool = pq.enter_context(tc.tile_pool(name="pq", bufs=1))
            qT_t = [q_pool.tile([P, T], BF16, tag=f"qT{i}", name=f"qT{i}") for i in range(NE)]

            p1 = ExitStack()
            x_pool = p1.enter_context(tc.tile_pool(name="px", bufs=1))
            w_pool = p1.enter_context(tc.tile_pool(name="pw", bufs=2))

            # ================= phase 1: projections =====================
            xT_t = [x_pool.tile([P, T], BF16, tag=f"xT{i}", name=f"xT{i}") for i in range(NE)]
            for e in range(NE):
                nc.sync.dma_start(xT_t[e][:], xT_d[e])
            xTk_t = [x_pool.tile([P, TK], BF16, tag=f"xTk{i}", name=f"xTk{i}") for i in range(NE)]
            for e in range(NE):
                nc.sync.dma_start(xTk_t[e][:], xTk_d[e])

            wq_t = w_pool.tile([P, NE, E], BF16, tag="w", name="wq_t")
            for e in range(NE):
                nc.sync.dma_start(wq_t[:, e, :], wq_d[e])
            wk_t = w_pool.tile([P, NE, E], BF16, tag="w", name="wk_t")
            for e in range(NE):
                nc.sync.dma_start(wk_t[:, e, :], wk_d[e])

            # qT[eo, t] = sum_e Wq[e, eo] * xT[e, t]   (full, replicated)
            for eo in range(NE):
                for tch in range(NQC):
                    ps = psum.tile([P, QCH], F32, tag="mm", name=f"ps_q{eo}_{tch}")
                    for e in range(NE):
                        nc.tensor.matmul(
                            ps[:],
                            wq_t[:, e, eo * P:(eo + 1) * P],
                            xT_t[e][:, tch * QCH:(tch + 1) * QCH],
                            start=(e == 0), stop=(e == NE - 1),
                        )
                    nc.scalar.copy(qT_t[eo][:, tch * QCH:(tch + 1) * QCH], ps[:])

            wv_t = w_pool.tile([P, NE, E], BF16, tag="w", name="wv_t")
            for e in range(NE):
                nc.sync.dma_start(wv_t[:, e, :], wv_d[e])

            # kTl[eo, kloc] = sum_e Wk[e, eo] * xTk[e, kloc]  (local slice)
            for eo in range(NE):
                ps = psum.tile([P, TK], F32, tag="mm", name=f"ps_k{eo}")
                for e in range(NE):
                    nc.tensor.matmul(
                        ps[:],
                        wk_t[:, e, eo * P:(eo + 1) * P],
                        xTk_t[e][:],
                        start=(e == 0), stop=(e == NE - 1),
                    )
                nc.scalar.copy(kTl_t[eo][:], ps[:])

            # Vl[kloc, e] = sum_e' xTk[e', kloc]^T * Wv[e', e]
            for tb in range(KL):
                for ec in range(E // QCH):
                    ps = psum.tile([P, QCH], F32, tag="mm", name=f"ps_v{tb}_{ec}")
                    for e in range(NE):
                        nc.tensor.matmul(
                            ps[:],
                            xTk_t[e][:, tb * P:(tb + 1) * P],
                            wv_t[:, e, ec * QCH:(ec + 1) * QCH],
                            start=(e == 0), stop=(e == NE - 1),
                        )
                    nc.scalar.copy(vl_t[tb][:, ec * QCH:(ec + 1) * QCH], ps[:])

            p1.close()  # frees xT/xTk + W pools

            # ============ phase 2: scores + softmax-over-q ==============
            # local tile j == global k-tile dv+4j; diagonal chunk qc = j
            p2 = ExitStack()
            wT_pool = p2.enter_context(tc.tile_pool(name="pwT", bufs=1, side="right"))
            wTl_t = [wT_pool.tile([P, T], BF16, tag=f"wTl{i}", name=f"wTl{i}") for i in range(KL)]

            for j in range(KL):
                for qc in range(j, NQC):
                    ps = psum.tile([P, QCH], F32, tag="mm", name=f"ps_s{j}_{qc}")
                    for e in range(NE):
                        nc.tensor.matmul(
                            ps[:],
                            kTl_t[e][:, j * P:(j + 1) * P],
                            qT_t[e][:, qc * QCH:(qc + 1) * QCH],
                            start=(e == 0), stop=(e == NE - 1),
                        )
                    wslice = wTl_t[j][:, qc * QCH:(qc + 1) * QCH]
                    acc = parts_t[:, j, qc:qc + 1]
                    if qc == j:
                        stg = stage.tile([P, QCH], F32, tag="stg", name=f"stg{j}")
                        nc.vector.tensor_add(stg[:], ps[:], mask_t[:])
                        nc.scalar.activation(wslice, stg[:], Exp, bias=0.0,
                                             scale=SCALE, accum_out=acc)
                    else:
                        nc.scalar.activation(wslice, ps[:], Exp, bias=0.0,
                                             scale=SCALE, accum_out=acc)
                nc.vector.reduce_sum(denom_t[:, j:j + 1], parts_t[:, j, j:NQC],
                                     axis=mybir.AxisListType.X)
                nc.vector.reciprocal(recip_t[:, j:j + 1], denom_t[:, j:j + 1])
                nc.vector.tensor_scalar_mul(vl_t[j][:], vl_t[j][:], recip_t[:, j:j + 1])

            pq.close()  # frees qT

            # ====== phase 3: partial outT[e,q] -> AllReduce =============
            po_stage = ExitStack()
            av_pool = po_stage.enter_context(tc.tile_pool(name="pav", bufs=4))
            for eb in range(NE):
                for qc in range(NQC):
                    ps = psum.tile([P, QCH], F32, tag="mm", name=f"ps_o{eb}_{qc}")
                    njs = qc + 1  # local tiles j <= qc contribute
                    for j in range(njs):
                        nc.tensor.matmul(
                            ps[:],
                            vl_t[j][:, eb * P:(eb + 1) * P],
                            wTl_t[j][:, qc * QCH:(qc + 1) * QCH],
                            start=(j == 0), stop=(j == njs - 1),
                        )
                    stg = av_pool.tile([P, QCH], BF16, tag="av", name=f"av{eb}_{qc}")
                    nc.scalar.copy(stg[:], ps[:])
                    nc.sync.dma_start(cc_in[eb, :, qc * QCH:(qc + 1) * QCH], stg[:])
            po_stage.close()
            p2.close()  # frees wTl

            nc.gpsimd.collective_compute(
                "AllReduce", mybir.AluOpType.add,
                replica_groups=groups,
                ins=[cc_in.opt()], outs=[cc_out.opt()],
            )

            # ============ phase 4: readout ==============================
            p3 = ExitStack()
            outT_pool = p3.enter_context(tc.tile_pool(name="poutT", bufs=1))
            outT_t = [outT_pool.tile([P, T], BF16, tag=f"oT{i}", name=f"oT{i}") for i in range(NE)]
            for e in range(NE):
                nc.sync.dma_start(outT_t[e][:], cc_out[e])

            p4 = ExitStack()
            ro_pool = p4.enter_context(tc.tile_pool(name="pro", bufs=2))
            ostg_pool = p4.enter_context(tc.tile_pool(name="postg", bufs=4))

            for vc in range(NVC):
                wro_t = ro_pool.tile([P, NE, VCH], BF16, tag="wro", name=f"wro{vc}")
                for e in range(NE):
                    nc.sync.dma_start(wro_t[:, e, :], wro_d[e, :, vc * VCH:(vc + 1) * VCH])
                for tb in range(NT):
                    ps = psum.tile([P, VCH], F32, tag="mm", name=f"ps_r{vc}_{tb}")
                    for e in range(NE):
                        nc.tensor.matmul(
                            ps[:],
                            outT_t[e][:, tb * P:(tb + 1) * P],
                            wro_t[:, e, :],
                            start=(e == 0), stop=(e == NE - 1),
                        )
                    stg = ostg_pool.tile([P, VCH], F32, tag="ostg", name=f"ostg{vc}_{tb}")
                    if tb % 2 == 0:
                        nc.vector.tensor_copy(stg[:], ps[:])
                    else:
                        nc.scalar.copy(stg[:], ps[:])
                    nc.sync.dma_start(out_d[tb, :, vc * VCH:(vc + 1) * VCH], stg[:])

            p4.close()
            p3.close()

    nc.compile()
    return nc


def _get_nc():
    if "nc" not in _CACHE:
        _CACHE["nc"] = _build_program()
    return _CACHE["nc"]


def _make_in_maps(X, emb_table, pos_table, Wk, Wq, Wv, Wro):
    bf = ml_dtypes.bfloat16
    X = np.asarray(X)
    emb_table = np.asarray(emb_table, np.float32)
    pos_table = np.asarray(pos_table, np.float32)

    # host-side embedding gather + positional add (0.03% of model FLOPs)
    x = emb_table[X] + pos_table[None, :, :]            # [B, T, E] f32

    wk = np.ascontiguousarray(np.asarray(Wk, np.float32).reshape(NE, P, E)).astype(bf)
    wq = np.ascontiguousarray(np.asarray(Wq, np.float32).reshape(NE, P, E)).astype(bf)
    wv = np.ascontiguousarray(np.asarray(Wv, np.float32).reshape(NE, P, E)).astype(bf)

    Wro = np.asarray(Wro, np.float32)
    wro_s = []
    for s in range(VSPLIT):
        sl = Wro[:, s * VS:(s + 1) * VS].reshape(NE, P, VS)
        wro_s.append(np.ascontiguousarray(sl).astype(bf))

    xT_b, xTk_b = [], []
    for b in range(B):
        xt = np.ascontiguousarray(x[b].T)                       # [E, T] f32
        xT_b.append(xt.reshape(NE, P, T).astype(bf))
        per_dv = []
        for dv in range(VSPLIT):
            cols = np.concatenate(
                [xt[:, (dv + 4 * j) * P:(dv + 4 * j + 1) * P] for j in range(KL)],
                axis=1,
            )                                                   # [E, TK]
            per_dv.append(np.ascontiguousarray(cols).reshape(NE, P, TK).astype(bf))
        xTk_b.append(per_dv)

    # staircase mask for the diagonal chunk of local tile j (global dv+4j):
    # masked iff col < dv*128 + p  — depends only on dv
    p_idx = np.arange(P)[:, None]
    c_idx = np.arange(QCH)[None, :]
    masks = [
        np.where(c_idx < dv * P + p_idx, MASK_VAL, 0.0).astype(np.float32)
        for dv in range(VSPLIT)
    ]

    in_maps = []
    for c in range(8):
        b, dv = divmod(c, VSPLIT)
        in_maps.append({
            "xT": xT_b[b],
            "xTk": xTk_b[b][dv],
            "wk": wk, "wq": wq, "wv": wv,
            "wro": wro_s[dv],
            "mask": masks[dv],
        })
    return in_maps


def run_on_device(in_maps, trace=False, **kw):
    nc = _get_nc()
    return run_bass_kernel_spmd(nc, in_maps, core_ids=list(range(8)), trace=trace, **kw)


def _unshard(results):
    """results: list of 8 per-core dicts -> full [B, T, VOC] logits.

    Core c owns batch c//4 and vocab slice c%4.  The attention-output
    AllReduce sums the interleaved k-tile partials, so per-core logits
    are already complete for their (batch, vocab-slice)."""
    logits = np.empty((B, T, VOC), np.float32)
    for c in range(8):
        b, s = divmod(c, VSPLIT)
        logits[b, :, s * VS:(s + 1) * VS] = results[c]["logits"].reshape(T, VS)
    return logits


def kernel(X, emb_table, pos_table, Wk, Wq, Wv, Wro, bro):
    in_maps = _make_in_maps(X, emb_table, pos_table, Wk, Wq, Wv, Wro)
    _CACHE["in_maps"] = in_maps

    res = run_on_device(in_maps, trace=False)
    _CACHE["last_results"] = res

    logits = _unshard(res.results)

    bro = np.asarray(bro, np.float32)
    if np.any(bro):
        logits += bro
    return logits
